# revision 42
# baseline (speedup 1.0000x reference)
"""AttentionalPooler Trainium2 kernel (v2, fp16 PE paths).

Data-parallel over batch: each of 8 NeuronCores processes one batch element
(x[i]: [4096, 1024], shipped to the device pre-cast to fp16).

Key design points vs the f32r baseline:
  - All matmul operands fp16: streams run 1 row/cycle (f32r ran ~2.6x slower
    with fp32_mode=HIGH double passes) and LDWEIGHTS is ~2x faster (+FWL).
  - LayerNorm is never applied to x. zrow = x - mean only; the 1/std factor
    is algebraically folded:
      * K path: logits scale per-key -> applied as the per-partition `scale`
        of the Exp activation (free).
      * V path: applied as per-partition scale in the PSUM->SBUF drain of the
        V projection (replaces a plain copy, free).
      * K bias is dropped entirely (constant per query -> softmax invariant).
  - rsqrt(var+eps) computed on the vector engine via Quake-style Newton
    iteration -> the scalar engine only ever runs EXP (no ACT table thrash).
  - K projection streams N=512 per weight load (chunk = 512 rows).
  - Attention pipeline interleaves head h's K matmuls with head h-1's sim and
    head h-2's attn@v so the scalar-engine Exp latency is hidden; the next
    chunk's transposes (or the finale) also slot into this pipeline.
  - Softmax denominator via an appended ones-column in v_aug; the [97, 256]
    per-head accumulators are transposed at the end so the normalization is a
    per-partition (per-query) tensor_scalar.
"""

import sys

for p in ("/opt/trn_rl_repo",):
    if p not in sys.path:
        sys.path.insert(0, p)

import numpy as np

import concourse.bass as bass
import concourse.tile as tile
from concourse import bacc
from concourse import mybir
from concourse.bass_utils import run_bass_kernel_spmd

F32 = mybir.dt.float32
F16 = mybir.dt.float16
I32 = mybir.dt.int32

N_CORES = 8
B, N, CTX = 8, 4096, 1024
NQ, DM, H = 256, 768, 8
DH = DM // H  # 96
EPS = 1e-5
CHUNK = 512
N_CHUNKS = N // CHUNK  # 8
RT = CHUNK // 128  # 4 row-tiles per chunk
CC = CTX // 128  # 8 contraction chunks
RSQRT_MAGIC = 0x5F3759DF


def emit_rsqrt(nc, pool, out, var, eps, tag):
    """out = 1/sqrt(var + eps), [128, n] f32, vector engine only."""
    n = out.shape[1]
    ve = pool.tile([128, n], F32, tag=f"{tag}_ve")
    nc.vector.tensor_scalar_add(out=ve, in0=var, scalar1=float(eps))
    sh = pool.tile([128, n], I32, tag=f"{tag}_sh")
    nc.vector.tensor_scalar(
        out=sh, in0=ve.bitcast(I32), scalar1=1, scalar2=None,
        op0=mybir.AluOpType.logical_shift_right,
    )
    magic = pool.tile([128, n], I32, tag=f"{tag}_mg")
    nc.vector.memset(magic, RSQRT_MAGIC)
    # y0 = bitcast(magic - (i >> 1))
    nc.vector.scalar_tensor_tensor(
        out=out.bitcast(I32), in0=magic, scalar=0, in1=sh,
        op0=mybir.AluOpType.bypass, op1=mybir.AluOpType.subtract,
    )
    t = pool.tile([128, n], F32, tag=f"{tag}_t")
    for _ in range(2):  # two Newton steps: rel err ~4e-6
        nc.vector.tensor_mul(out=t, in0=out, in1=out)
        nc.vector.tensor_mul(out=t, in0=t, in1=ve)
        nc.vector.tensor_scalar(
            out=t, in0=t, scalar1=-0.5, scalar2=1.5,
            op0=mybir.AluOpType.mult, op1=mybir.AluOpType.add,
        )
        nc.vector.tensor_mul(out=out, in0=out, in1=t)


def build_nc(has_out_bias):
    nc = bacc.Bacc("TRN2", debug=False)
    x = nc.dram_tensor("x", [N, CTX], F16, kind="ExternalInput")
    query = nc.dram_tensor("query", [NQ, DM], F16, kind="ExternalInput")
    wkvg = nc.dram_tensor("wkvg", [CTX, 2 * DM], F16, kind="ExternalInput")
    wqg = nc.dram_tensor("wqg", [DM, DM], F16, kind="ExternalInput")
    wout = nc.dram_tensor("wout", [DM, DM], F16, kind="ExternalInput")
    qbias = nc.dram_tensor("qbias", [DM], F32, kind="ExternalInput")
    identh_p = nc.dram_tensor("identh", [128, 128], F16, kind="ExternalInput")
    identf_p = nc.dram_tensor("identf", [128, 128], F32, kind="ExternalInput")
    if has_out_bias:
        out_bias = nc.dram_tensor("out_bias", [1, DM], F32, kind="ExternalInput")
    out = nc.dram_tensor("out", [NQ, DM], F32, kind="ExternalOutput")

    from contextlib import ExitStack

    with tile.TileContext(nc) as tc, ExitStack() as es:
        singles = es.enter_context(tc.tile_pool(name="singles", bufs=1))
        work = es.enter_context(tc.tile_pool(name="work", bufs=3))
        xpool = es.enter_context(tc.tile_pool(name="xpool", bufs=9))
        chunkp = es.enter_context(tc.tile_pool(name="chunkp", bufs=2))
        attp = es.enter_context(tc.tile_pool(name="attp", bufs=4))
        # PSUM pools: bank-granular; 1+1+2+3+1 = 8 banks exactly
        pz_pool = es.enter_context(tc.tile_pool(name="pz", bufs=1, space="PSUM"))
        pk_pool = es.enter_context(tc.tile_pool(name="pk", bufs=1, space="PSUM"))
        pv_pool = es.enter_context(tc.tile_pool(name="pv", bufs=2, space="PSUM"))
        ps_pool = es.enter_context(tc.tile_pool(name="ps", bufs=3, space="PSUM"))
        pa_pool = es.enter_context(tc.tile_pool(name="pa", bufs=1, space="PSUM"))

        # ---- DMAs: x + wkv on the SP queue, q-path on the ACT queue (parallel)
        identh = singles.tile([128, 128], F16)
        nc.scalar.dma_start(out=identh, in_=identh_p[:, :])
        qtiles = singles.tile([128, 2, DM], F16)
        nc.scalar.dma_start(out=qtiles[:, 0, :], in_=query[0:128, :])
        nc.scalar.dma_start(out=qtiles[:, 1, :], in_=query[128:256, :])
        wq_sb = singles.tile([128, DM // 128, DM], F16)
        nc.scalar.dma_start(out=wq_sb, in_=wqg.rearrange("(cc p) j -> p cc j", p=128))
        qb_sb = singles.tile([DH, H], F32)
        nc.scalar.dma_start(out=qb_sb, in_=qbias.rearrange("(h p) -> p h", p=DH))
        identf = singles.tile([128, 128], F32)
        nc.scalar.dma_start(out=identf, in_=identf_p[:, :])
        xts0 = []
        for rt in range(RT):
            xt = xpool.tile([128, CTX], F16, tag="xt")
            nc.sync.dma_start(out=xt, in_=x[rt * 128 : (rt + 1) * 128, :])
            xts0.append(xt)
        # V-half of the fused KV weights first: it gates V(0); x1 before the
        # K-half (transposes of chunk 1 are needed before K(0) finishes)
        wkv_sb = singles.tile([128, CC, 2 * DM], F16)
        wkv_re = wkvg.rearrange("(cc p) j -> p cc j", p=128)
        nc.sync.dma_start(out=wkv_sb[:, :, DM:], in_=wkv_re[:, :, DM:])
        xts1 = []
        for rt in range(RT):
            xt = xpool.tile([128, CTX], F16, tag="xt")
            nc.sync.dma_start(out=xt, in_=x[CHUNK + rt * 128 : CHUNK + (rt + 1) * 128, :])
            xts1.append(xt)
        nc.sync.dma_start(out=wkv_sb[:, :, 0:DM], in_=wkv_re[:, :, 0:DM])
        wout_sb = singles.tile([DH, H, DM], F16)

        acc = singles.tile([DH + 1, H, NQ], F32)
        nc.vector.memset(acc, 0.0)
        eps_t = singles.tile([128, 1], F32)
        nc.vector.memset(eps_t, EPS)
        qT = singles.tile([DH, H, NQ], F16)
        pooledT = singles.tile([DH, H, NQ], F16)

        # ---- query path (PE work while x/wkv DMAs stream in) ----
        zqT = singles.tile([128, DM // 128, NQ], F16)
        for qt in range(2):
            qtile = qtiles[:, qt, :]
            st = work.tile([128, 2, 6], F32, tag="qstats")
            qv = qtile.rearrange("p (s d) -> p s d", s=2)
            for s in range(2):
                nc.vector.bn_stats(out=st[:, s, :], in_=qv[:, s, :])
            mv = work.tile([128, 2], F32, tag="qmv")
            nc.vector.bn_aggr(out=mv, in_=st)
            # startup-latency path: scalar Sqrt (table loads once, before any
            # Exp) + vector reciprocal beats the 15-op Newton chain's latency
            qstd = work.tile([128, 1], F32, tag="qstd")
            nc.scalar.activation(
                out=qstd, in_=mv[:, 1:2],
                func=mybir.ActivationFunctionType.Sqrt, bias=eps_t,
            )
            qrstd = work.tile([128, 1], F32, tag="qrstd")
            nc.vector.reciprocal(out=qrstd, in_=qstd)
            zq = work.tile([128, DM], F16, tag="zq")
            nc.vector.tensor_scalar(
                out=zq, in0=qtile,
                scalar1=mv[:, 0:1], scalar2=qrstd,
                op0=mybir.AluOpType.subtract, op1=mybir.AluOpType.mult,
            )
            pt1 = pz_pool.tile([128, 4, 128], F16, tag="pz")
            for i in range(4):
                nc.tensor.transpose(
                    pt1[:, i, :], zq[:, i * 128 : (i + 1) * 128], identh
                )
            nc.vector.tensor_copy(
                out=zqT[:, 0:4, qt * 128 : (qt + 1) * 128], in_=pt1
            )
            pt2 = pz_pool.tile([128, 4, 128], F16, tag="pz")
            for i in range(2):
                nc.tensor.transpose(
                    pt2[:, i, :], zq[:, (4 + i) * 128 : (5 + i) * 128], identh
                )
            nc.vector.tensor_copy(
                out=zqT[:, 4:6, qt * 128 : (qt + 1) * 128], in_=pt2[:, 0:2, :]
            )
        for h in range(H):
            pq = ps_pool.tile([128, NQ], F32, tag="ps")
            for cc in range(DM // 128):
                nc.tensor.matmul(
                    pq[0:DH, :],
                    wq_sb[:, cc, h * DH : (h + 1) * DH],
                    zqT[:, cc, :],
                    start=(cc == 0), stop=(cc == DM // 128 - 1),
                )
            nc.vector.tensor_scalar_add(
                out=qT[:, h, :], in0=pq[0:DH, :], scalar1=qb_sb[:, h : h + 1]
            )

        # ---- software-pipelined main loop ----
        zrows, rstds, zTs, vaugs = {}, {}, {}, {}
        ln_state = {}

        def emit_ln_alloc(c, xts=None, fast_rstd=False):
            if xts is None:
                xts = []
                for rt in range(RT):
                    xt = xpool.tile([128, CTX], F16, tag="xt")
                    r0 = c * CHUNK + rt * 128
                    nc.sync.dma_start(out=xt, in_=x[r0 : r0 + 128, :])
                    xts.append(xt)
            mv_all = chunkp.tile([128, RT, 2], F32, tag="mv")
            zrow4 = chunkp.tile([128, RT, CTX], F16, tag="zrow")
            rstd4 = chunkp.tile([128, RT], F32, tag="rstd")
            zT_c = chunkp.tile([128, CC, CHUNK], F16, tag="zT")
            ln_state[c] = (xts, mv_all, rstd4, fast_rstd)
            zrows[c] = zrow4
            zTs[c] = zT_c

        def emit_ln_stats(c, rt):
            xts, mv_all, _, _ = ln_state[c]
            st = work.tile([128, 2, 6], F32, tag="xstats")
            xv = xts[rt].rearrange("p (s d) -> p s d", s=2)
            for s in range(2):
                nc.vector.bn_stats(out=st[:, s, :], in_=xv[:, s, :])
            nc.vector.bn_aggr(out=mv_all[:, rt, :], in_=st)

        def emit_ln_rstd(c):
            _, mv_all, rstd4, fast = ln_state[c]
            if fast:
                xstd = work.tile([128, RT], F32, tag="xstd")
                nc.scalar.activation(
                    out=xstd, in_=mv_all[:, :, 1],
                    func=mybir.ActivationFunctionType.Sqrt, bias=eps_t,
                )
                nc.vector.reciprocal(out=rstd4, in_=xstd)
            else:
                emit_rsqrt(nc, work, rstd4, mv_all[:, :, 1], EPS, tag="xr")

        def emit_ln_zrow(c, rt):
            xts, mv_all, rstd4, _ = ln_state[c]
            nc.vector.tensor_scalar(
                out=zrows[c][:, rt, :], in0=xts[rt],
                scalar1=mv_all[:, rt, 0:1], scalar2=rstd4[:, rt : rt + 1],
                op0=mybir.AluOpType.subtract, op1=mybir.AluOpType.mult,
            )

        def emit_load_ln(c, xts=None, fast_rstd=False):
            emit_ln_alloc(c, xts=xts, fast_rstd=fast_rstd)
            for rt in range(RT):
                emit_ln_stats(c, rt)
            emit_ln_rstd(c)
            for rt in range(RT):
                emit_ln_zrow(c, rt)

        def transpose_slices(c):
            """8 closures, each: 4 transposes + 1 drain into zT(c)."""
            zrow4, zT = zrows[c], zTs[c]
            out = []
            for rt in range(RT):
                for ccp in range(2):
                    def emit(rt=rt, ccp=ccp):
                        pt = pz_pool.tile([128, 4, 128], F16, tag="pz")
                        for i in range(4):
                            cc = ccp * 4 + i
                            nc.tensor.transpose(
                                pt[:, i, :],
                                zrow4[:, rt, cc * 128 : (cc + 1) * 128],
                                identh,
                            )
                        nc.vector.tensor_copy(
                            out=zT[:, ccp * 4 : (ccp + 1) * 4,
                                   rt * 128 : (rt + 1) * 128],
                            in_=pt,
                        )
                    out.append(emit)
            return out

        def emit_v_proj(c):
            zT = zTs[c]
            v_aug = chunkp.tile([128, RT, H, DH + 1], F16, tag="vaug")
            for rt in range(RT):
                for vh in range(2):
                    pv = pv_pool.tile([128, 384], F32, tag="pv")
                    j0 = DM + vh * 384
                    for cc in range(CC):
                        nc.tensor.matmul(
                            pv, zT[:, cc, rt * 128 : (rt + 1) * 128],
                            wkv_sb[:, cc, j0 : j0 + 384],
                            start=(cc == 0), stop=(cc == CC - 1),
                        )
                    nc.vector.tensor_copy(
                        out=v_aug[:, rt, vh * 4 : (vh + 1) * 4, 0:DH],
                        in_=pv.rearrange("p (h d) -> p h d", d=DH),
                    )
            nc.vector.memset(v_aug[:, :, :, DH : DH + 1], 1.0)
            vaugs[c] = v_aug

        def emit_k_head(c, h):
            zT = zTs[c]
            pk = pk_pool.tile([DH, CHUNK], F32, tag="pk")
            for cc in range(CC):
                nc.tensor.matmul(
                    pk, wkv_sb[:, cc, h * DH : (h + 1) * DH], zT[:, cc, :],
                    start=(cc == 0), stop=(cc == CC - 1),
                )
            kT = attp.tile([DH, CHUNK], F16, tag="kT")
            nc.vector.tensor_copy(out=kT, in_=pk)
            return kT

        def emit_sim_pair(c, h, kT, at4, rtp):
            for i in range(2):
                rt = rtp * 2 + i
                ps = ps_pool.tile([128, NQ], F32, tag="ps")
                nc.tensor.matmul(
                    ps, kT[:, rt * 128 : (rt + 1) * 128], qT[:, h, :],
                    start=True, stop=True,
                )
                nc.scalar.activation(
                    out=at4[:, rt, :], in_=ps,
                    func=mybir.ActivationFunctionType.Exp,
                )

        def emit_pv_head(c, h, at4):
            v_aug = vaugs[c]
            pacc = pa_pool.tile([DH + 1, NQ], F32, tag="pa")
            for rt in range(RT):
                nc.tensor.matmul(
                    pacc, v_aug[:, rt, h, :], at4[:, rt, :],
                    start=(rt == 0), stop=(rt == RT - 1),
                )
            nc.vector.tensor_add(out=acc[:, h, :], in0=acc[:, h, :], in1=pacc)

        def emit_finale_head(h):
            # acc[:, h, :] -> per-query normalize -> pooledT[:, h, :]
            for qh in range(2):
                ps = ps_pool.tile([128, NQ], F32, tag="ps")
                nc.tensor.transpose(
                    ps[:, 0 : DH + 1],
                    acc[:, h, qh * 128 : (qh + 1) * 128],
                    identf[0 : DH + 1, 0 : DH + 1],
                )
                rz = work.tile([128, 1], F32, tag="rz")
                nc.vector.reciprocal(out=rz, in_=ps[:, DH : DH + 1])
                pN = work.tile([128, DH], F16, tag="pN")
                nc.vector.tensor_scalar(
                    out=pN, in0=ps[:, 0:DH], scalar1=rz, scalar2=None,
                    op0=mybir.AluOpType.mult,
                )
                pt = pz_pool.tile([128, 4, 128], F16, tag="pz")
                nc.tensor.transpose(pt[0:DH, 0, :], pN, identh)
                nc.vector.tensor_copy(
                    out=pooledT[:, h, qh * 128 : (qh + 1) * 128],
                    in_=pt[0:DH, 0, :],
                )

        for ch in range(N_CHUNKS + 1):
            if ch == 0:
                emit_load_ln(0, xts=xts0, fast_rstd=True)
                for emit in transpose_slices(0):
                    emit()
                continue
            c = ch - 1
            # V projection first so its PSUM drains lead the vector queue
            # (they gate attn@v). The NEXT chunk's LN vector work is dripped
            # into the head loop so kT drains aren't queued behind it.
            emit_v_proj(c)
            if ch < N_CHUNKS:
                emit_ln_alloc(ch, xts=xts1 if ch == 1 else None)
            if ch == 2:
                nc.scalar.dma_start(
                    out=wout_sb, in_=wout.rearrange("(h p) j -> p h j", p=DH)
                )
            filler = transpose_slices(ch) if ch < N_CHUNKS else None
            if filler is None:
                # qh=0 output projection accumulates inline as heads finalize
                po0a = pv_pool.tile([128, 384], F32, tag="pv")
                po0b = pv_pool.tile([128, 384], F32, tag="pv")
                po0 = [po0a, po0b]
            fi = 0
            kTs, at4s = {}, {}
            for h in range(H + 2):
                # K(h) first: gives kT(h-1)'s drain a full PE-burst of slack.
                # sims for h-1 split around pv(h-2) so the psum slots
                # recycle only after the matching Exp has drained them.
                if h < H:
                    kTs[h] = emit_k_head(c, h)
                if 1 <= h <= H:
                    kT_prev = kTs.pop(h - 1)
                    at4 = attp.tile([128, RT, NQ], F16, tag="at")
                    at4s[h - 1] = at4
                    emit_sim_pair(c, h - 1, kT_prev, at4, 0)
                if h >= 2:
                    emit_pv_head(c, h - 2, at4s.pop(h - 2))
                    if filler is None:
                        emit_finale_head(h - 2)
                        for j in range(2):
                            nc.tensor.matmul(
                                po0[j], pooledT[:, h - 2, 0:128],
                                wout_sb[:, h - 2, j * 384 : (j + 1) * 384],
                                start=(h - 2 == 0), stop=(h - 2 == H - 1),
                            )
                if 1 <= h <= H:
                    emit_sim_pair(c, h - 1, kT_prev, at4s[h - 1], 1)
                if filler is not None:
                    # drip next chunk's LN: stats at h<4, rstd at 4, zrow 4-7,
                    # transposes (PE fillers) at 5-8 chasing zrow readiness
                    if h < RT:
                        emit_ln_stats(ch, h)
                    if h == RT:
                        emit_ln_rstd(ch)
                    if RT <= h < 2 * RT:
                        emit_ln_zrow(ch, h - RT)
                    if 5 <= h <= 8:
                        for _ in range(2):
                            if fi < len(filler):
                                filler[fi](); fi += 1
            if filler is not None:
                while fi < len(filler):
                    filler[fi](); fi += 1

        # ---- output projection ----
        ob_bc = None
        if has_out_bias:
            ob = singles.tile([1, DM], F32)
            nc.sync.dma_start(out=ob, in_=out_bias[:, :])
            ob_bc = singles.tile([128, DM], F32)
            nc.gpsimd.partition_broadcast(ob_bc, ob)
        for qh in range(2):
            if qh == 0:
                pj = po0  # accumulated inline during the finale
            else:
                pja = pv_pool.tile([128, 384], F32, tag="pv")
                pjb = pv_pool.tile([128, 384], F32, tag="pv")
                pj = [pja, pjb]
                for h in range(H):
                    for j in range(2):
                        nc.tensor.matmul(
                            pj[j], pooledT[:, h, qh * 128 : (qh + 1) * 128],
                            wout_sb[:, h, j * 384 : (j + 1) * 384],
                            start=(h == 0), stop=(h == H - 1),
                        )
            ot = work.tile([128, DM], F32, tag="ot")
            for j in range(2):
                sl = slice(j * 384, (j + 1) * 384)
                if ob_bc is not None:
                    nc.vector.tensor_add(out=ot[:, sl], in0=pj[j], in1=ob_bc[:, sl])
                else:
                    nc.vector.tensor_copy(out=ot[:, sl], in_=pj[j])
            nc.sync.dma_start(out=out[qh * 128 : (qh + 1) * 128, :], in_=ot)
    nc.compile()
    return nc


_NC_CACHE = {}
_TRACE = False


def kernel(**inputs):
    x = np.asarray(inputs["x"], dtype=np.float32)
    query = np.asarray(inputs["query"], dtype=np.float32)
    ln_k_g = np.asarray(inputs["ln_k_g"], dtype=np.float32)
    ln_k_b = np.asarray(inputs["ln_k_b"], dtype=np.float32)
    ln_q_g = np.asarray(inputs["ln_q_g"], dtype=np.float32)
    ln_q_b = np.asarray(inputs["ln_q_b"], dtype=np.float32)
    W_q = np.asarray(inputs["W_q"], dtype=np.float32)
    W_kv = np.asarray(inputs["W_kv"], dtype=np.float32)
    W_out = np.asarray(inputs["W_out"], dtype=np.float32)

    scale = DH ** -0.5
    wkvg = (ln_k_g[:, None] * W_kv).astype(np.float16)
    wqg = (ln_q_g[:, None] * W_q * scale).astype(np.float16)
    qbias = ((ln_q_b @ W_q) * scale).astype(np.float32)
    kv_bias = ln_k_b @ W_kv
    vb = kv_bias[DM:]
    has_out_bias = bool(np.any(vb != 0.0))
    x16 = x.astype(np.float16)

    key = has_out_bias
    if key not in _NC_CACHE:
        _NC_CACHE[key] = build_nc(has_out_bias)
    nc = _NC_CACHE[key]

    shared = dict(
        query=query.astype(np.float16), wkvg=wkvg, wqg=wqg,
        wout=W_out.astype(np.float16),
        qbias=qbias,
        identh=np.eye(128, dtype=np.float16),
        identf=np.eye(128, dtype=np.float32),
    )
    if has_out_bias:
        shared["out_bias"] = (vb @ W_out).astype(np.float32).reshape(1, DM)
    in_maps = [dict(x=x16[i], **shared) for i in range(N_CORES)]
    res = run_bass_kernel_spmd(
        nc, in_maps, core_ids=list(range(N_CORES)), trace=_TRACE
    )
    kernel.last_result = res
    out = np.stack([np.asarray(res.results[i]["out"]) for i in range(N_CORES)])
    return out.astype(np.float32)


if __name__ == "__main__":
    rng = np.random.default_rng(0)
    ins = {
        "x": rng.standard_normal((B, N, CTX), dtype=np.float32),
        "query": rng.standard_normal((NQ, DM), dtype=np.float32),
        "ln_k_g": np.ones(CTX, np.float32),
        "ln_k_b": np.zeros(CTX, np.float32),
        "ln_q_g": np.ones(DM, np.float32),
        "ln_q_b": np.zeros(DM, np.float32),
        "W_q": rng.standard_normal((DM, DM), dtype=np.float32) * DM ** -0.5,
        "W_kv": rng.standard_normal((CTX, 2 * DM), dtype=np.float32) * CTX ** -0.5,
        "W_out": rng.standard_normal((DM, DM), dtype=np.float32) * DM ** -0.5,
    }
    o = kernel(**ins)
    print("out", o.shape, o.dtype, float(np.abs(o).mean()))


# revision 44
# speedup vs baseline: 1.0114x; 1.0114x over previous
"""AttentionalPooler Trainium2 kernel (v2, fp16 PE paths).

Data-parallel over batch: each of 8 NeuronCores processes one batch element
(x[i]: [4096, 1024], shipped to the device pre-cast to fp16).

Key design points vs the f32r baseline:
  - All matmul operands fp16: streams run 1 row/cycle (f32r ran ~2.6x slower
    with fp32_mode=HIGH double passes) and LDWEIGHTS is ~2x faster (+FWL).
  - LayerNorm is never applied to x. zrow = x - mean only; the 1/std factor
    is algebraically folded:
      * K path: logits scale per-key -> applied as the per-partition `scale`
        of the Exp activation (free).
      * V path: applied as per-partition scale in the PSUM->SBUF drain of the
        V projection (replaces a plain copy, free).
      * K bias is dropped entirely (constant per query -> softmax invariant).
  - rsqrt(var+eps) computed on the vector engine via Quake-style Newton
    iteration -> the scalar engine only ever runs EXP (no ACT table thrash).
  - K projection streams N=512 per weight load (chunk = 512 rows).
  - Attention pipeline interleaves head h's K matmuls with head h-1's sim and
    head h-2's attn@v so the scalar-engine Exp latency is hidden; the next
    chunk's transposes (or the finale) also slot into this pipeline.
  - Softmax denominator via an appended ones-column in v_aug; the [97, 256]
    per-head accumulators are transposed at the end so the normalization is a
    per-partition (per-query) tensor_scalar.
"""

import sys

for p in ("/opt/trn_rl_repo",):
    if p not in sys.path:
        sys.path.insert(0, p)

import numpy as np

import concourse.bass as bass
import concourse.tile as tile
from concourse import bacc
from concourse import mybir
from concourse.bass_utils import run_bass_kernel_spmd

F32 = mybir.dt.float32
F16 = mybir.dt.float16
I32 = mybir.dt.int32

N_CORES = 8
B, N, CTX = 8, 4096, 1024
NQ, DM, H = 256, 768, 8
DH = DM // H  # 96
EPS = 1e-5
CHUNK = 512
N_CHUNKS = N // CHUNK  # 8
RT = CHUNK // 128  # 4 row-tiles per chunk
CC = CTX // 128  # 8 contraction chunks
RSQRT_MAGIC = 0x5F3759DF


def emit_rsqrt(nc, pool, out, var, eps, tag):
    """out = 1/sqrt(var + eps), [128, n] f32, vector engine only."""
    n = out.shape[1]
    ve = pool.tile([128, n], F32, tag=f"{tag}_ve")
    nc.vector.tensor_scalar_add(out=ve, in0=var, scalar1=float(eps))
    sh = pool.tile([128, n], I32, tag=f"{tag}_sh")
    nc.vector.tensor_scalar(
        out=sh, in0=ve.bitcast(I32), scalar1=1, scalar2=None,
        op0=mybir.AluOpType.logical_shift_right,
    )
    magic = pool.tile([128, n], I32, tag=f"{tag}_mg")
    nc.vector.memset(magic, RSQRT_MAGIC)
    # y0 = bitcast(magic - (i >> 1))
    nc.vector.scalar_tensor_tensor(
        out=out.bitcast(I32), in0=magic, scalar=0, in1=sh,
        op0=mybir.AluOpType.bypass, op1=mybir.AluOpType.subtract,
    )
    t = pool.tile([128, n], F32, tag=f"{tag}_t")
    for _ in range(2):  # two Newton steps: rel err ~4e-6
        nc.vector.tensor_mul(out=t, in0=out, in1=out)
        nc.vector.tensor_mul(out=t, in0=t, in1=ve)
        nc.vector.tensor_scalar(
            out=t, in0=t, scalar1=-0.5, scalar2=1.5,
            op0=mybir.AluOpType.mult, op1=mybir.AluOpType.add,
        )
        nc.vector.tensor_mul(out=out, in0=out, in1=t)


def build_nc(has_out_bias):
    nc = bacc.Bacc("TRN2", debug=False)
    x = nc.dram_tensor("x", [N, CTX], F16, kind="ExternalInput")
    query = nc.dram_tensor("query", [NQ, DM], F16, kind="ExternalInput")
    wkvg = nc.dram_tensor("wkvg", [CTX, 2 * DM], F16, kind="ExternalInput")
    wqg = nc.dram_tensor("wqg", [DM, DM], F16, kind="ExternalInput")
    wout = nc.dram_tensor("wout", [DM, DM], F16, kind="ExternalInput")
    qbias = nc.dram_tensor("qbias", [DM], F32, kind="ExternalInput")
    identh_p = nc.dram_tensor("identh", [128, 128], F16, kind="ExternalInput")
    identf_p = nc.dram_tensor("identf", [128, 128], F32, kind="ExternalInput")
    if has_out_bias:
        out_bias = nc.dram_tensor("out_bias", [1, DM], F32, kind="ExternalInput")
    out = nc.dram_tensor("out", [NQ, DM], F32, kind="ExternalOutput")

    from contextlib import ExitStack

    with tile.TileContext(nc) as tc, ExitStack() as es:
        singles = es.enter_context(tc.tile_pool(name="singles", bufs=1))
        work = es.enter_context(tc.tile_pool(name="work", bufs=3))
        xpool = es.enter_context(tc.tile_pool(name="xpool", bufs=9))
        chunkp = es.enter_context(tc.tile_pool(name="chunkp", bufs=2))
        attp = es.enter_context(tc.tile_pool(name="attp", bufs=4))
        # PSUM pools: bank-granular; 1+1+2+3+1 = 8 banks exactly
        pz_pool = es.enter_context(tc.tile_pool(name="pz", bufs=1, space="PSUM"))
        pk_pool = es.enter_context(tc.tile_pool(name="pk", bufs=1, space="PSUM"))
        pv_pool = es.enter_context(tc.tile_pool(name="pv", bufs=2, space="PSUM"))
        ps_pool = es.enter_context(tc.tile_pool(name="ps", bufs=3, space="PSUM"))
        pa_pool = es.enter_context(tc.tile_pool(name="pa", bufs=1, space="PSUM"))

        # ---- DMAs: x + wkv on the SP queue, q-path on the ACT queue (parallel)
        identh = singles.tile([128, 128], F16)
        nc.scalar.dma_start(out=identh, in_=identh_p[:, :])
        qtiles = singles.tile([128, 2, DM], F16)
        nc.scalar.dma_start(out=qtiles[:, 0, :], in_=query[0:128, :])
        nc.scalar.dma_start(out=qtiles[:, 1, :], in_=query[128:256, :])
        wq_sb = singles.tile([128, DM // 128, DM], F16)
        nc.scalar.dma_start(out=wq_sb, in_=wqg.rearrange("(cc p) j -> p cc j", p=128))
        qb_sb = singles.tile([DH, H], F32)
        nc.scalar.dma_start(out=qb_sb, in_=qbias.rearrange("(h p) -> p h", p=DH))
        identf = singles.tile([128, 128], F32)
        nc.scalar.dma_start(out=identf, in_=identf_p[:, :])
        xts0 = []
        for rt in range(RT):
            xt = xpool.tile([128, CTX], F16, tag="xt")
            nc.sync.dma_start(out=xt, in_=x[rt * 128 : (rt + 1) * 128, :])
            xts0.append(xt)
        # V-half of the fused KV weights first: it gates V(0); x1 before the
        # K-half (transposes of chunk 1 are needed before K(0) finishes)
        wkv_sb = singles.tile([128, CC, 2 * DM], F16)
        wkv_re = wkvg.rearrange("(cc p) j -> p cc j", p=128)
        nc.sync.dma_start(out=wkv_sb[:, :, DM:], in_=wkv_re[:, :, DM:])
        nc.sync.dma_start(out=wkv_sb[:, :, 0:DM], in_=wkv_re[:, :, 0:DM])
        xts1 = []
        for rt in range(RT):
            xt = xpool.tile([128, CTX], F16, tag="xt")
            nc.sync.dma_start(out=xt, in_=x[CHUNK + rt * 128 : CHUNK + (rt + 1) * 128, :])
            xts1.append(xt)
        wout_sb = singles.tile([DH, H, DM], F16)

        acc = singles.tile([DH + 1, H, NQ], F32)
        nc.vector.memset(acc, 0.0)
        eps_t = singles.tile([128, 1], F32)
        nc.vector.memset(eps_t, EPS)
        qT = singles.tile([DH, H, NQ], F16)
        pooledT = singles.tile([DH, H, NQ], F16)

        # ---- query path (PE work while x/wkv DMAs stream in) ----
        zqT = singles.tile([128, DM // 128, NQ], F16)
        for qt in range(2):
            qtile = qtiles[:, qt, :]
            st = work.tile([128, 2, 6], F32, tag="qstats")
            qv = qtile.rearrange("p (s d) -> p s d", s=2)
            for s in range(2):
                nc.vector.bn_stats(out=st[:, s, :], in_=qv[:, s, :])
            mv = work.tile([128, 2], F32, tag="qmv")
            nc.vector.bn_aggr(out=mv, in_=st)
            # startup-latency path: scalar Sqrt (table loads once, before any
            # Exp) + vector reciprocal beats the 15-op Newton chain's latency
            qstd = work.tile([128, 1], F32, tag="qstd")
            nc.scalar.activation(
                out=qstd, in_=mv[:, 1:2],
                func=mybir.ActivationFunctionType.Sqrt, bias=eps_t,
            )
            qrstd = work.tile([128, 1], F32, tag="qrstd")
            nc.vector.reciprocal(out=qrstd, in_=qstd)
            zq = work.tile([128, DM], F16, tag="zq")
            nc.vector.tensor_scalar(
                out=zq, in0=qtile,
                scalar1=mv[:, 0:1], scalar2=qrstd,
                op0=mybir.AluOpType.subtract, op1=mybir.AluOpType.mult,
            )
            pt1 = pz_pool.tile([128, 4, 128], F16, tag="pz")
            for i in range(4):
                nc.tensor.transpose(
                    pt1[:, i, :], zq[:, i * 128 : (i + 1) * 128], identh
                )
            nc.vector.tensor_copy(
                out=zqT[:, 0:4, qt * 128 : (qt + 1) * 128], in_=pt1
            )
            pt2 = pz_pool.tile([128, 4, 128], F16, tag="pz")
            for i in range(2):
                nc.tensor.transpose(
                    pt2[:, i, :], zq[:, (4 + i) * 128 : (5 + i) * 128], identh
                )
            nc.vector.tensor_copy(
                out=zqT[:, 4:6, qt * 128 : (qt + 1) * 128], in_=pt2[:, 0:2, :]
            )
        for h in range(H):
            pq = ps_pool.tile([128, NQ], F32, tag="ps")
            for cc in range(DM // 128):
                nc.tensor.matmul(
                    pq[0:DH, :],
                    wq_sb[:, cc, h * DH : (h + 1) * DH],
                    zqT[:, cc, :],
                    start=(cc == 0), stop=(cc == DM // 128 - 1),
                )
            nc.vector.tensor_scalar_add(
                out=qT[:, h, :], in0=pq[0:DH, :], scalar1=qb_sb[:, h : h + 1]
            )

        # ---- software-pipelined main loop ----
        zrows, rstds, zTs, vaugs = {}, {}, {}, {}
        ln_state = {}

        def emit_ln_alloc(c, xts=None, fast_rstd=False):
            if xts is None:
                xts = []
                for rt in range(RT):
                    xt = xpool.tile([128, CTX], F16, tag="xt")
                    r0 = c * CHUNK + rt * 128
                    nc.sync.dma_start(out=xt, in_=x[r0 : r0 + 128, :])
                    xts.append(xt)
            mv_all = chunkp.tile([128, RT, 2], F32, tag="mv")
            zrow4 = chunkp.tile([128, RT, CTX], F16, tag="zrow")
            rstd4 = chunkp.tile([128, RT], F32, tag="rstd")
            zT_c = chunkp.tile([128, CC, CHUNK], F16, tag="zT")
            ln_state[c] = (xts, mv_all, rstd4, fast_rstd)
            zrows[c] = zrow4
            zTs[c] = zT_c

        def emit_ln_stats(c, rt):
            xts, mv_all, _, _ = ln_state[c]
            st = work.tile([128, 2, 6], F32, tag="xstats")
            xv = xts[rt].rearrange("p (s d) -> p s d", s=2)
            for s in range(2):
                nc.vector.bn_stats(out=st[:, s, :], in_=xv[:, s, :])
            nc.vector.bn_aggr(out=mv_all[:, rt, :], in_=st)

        def emit_ln_rstd(c):
            _, mv_all, rstd4, fast = ln_state[c]
            if fast:
                xstd = work.tile([128, RT], F32, tag="xstd")
                nc.scalar.activation(
                    out=xstd, in_=mv_all[:, :, 1],
                    func=mybir.ActivationFunctionType.Sqrt, bias=eps_t,
                )
                nc.vector.reciprocal(out=rstd4, in_=xstd)
            else:
                emit_rsqrt(nc, work, rstd4, mv_all[:, :, 1], EPS, tag="xr")

        def emit_ln_zrow(c, rt):
            xts, mv_all, rstd4, _ = ln_state[c]
            nc.vector.tensor_scalar(
                out=zrows[c][:, rt, :], in0=xts[rt],
                scalar1=mv_all[:, rt, 0:1], scalar2=rstd4[:, rt : rt + 1],
                op0=mybir.AluOpType.subtract, op1=mybir.AluOpType.mult,
            )

        def emit_load_ln(c, xts=None, fast_rstd=False):
            emit_ln_alloc(c, xts=xts, fast_rstd=fast_rstd)
            for rt in range(RT):
                emit_ln_stats(c, rt)
            emit_ln_rstd(c)
            for rt in range(RT):
                emit_ln_zrow(c, rt)

        def transpose_slices(c):
            """8 closures, each: 4 transposes + 1 drain into zT(c)."""
            zrow4, zT = zrows[c], zTs[c]
            out = []
            for rt in range(RT):
                for ccp in range(2):
                    def emit(rt=rt, ccp=ccp):
                        pt = pz_pool.tile([128, 4, 128], F16, tag="pz")
                        for i in range(4):
                            cc = ccp * 4 + i
                            nc.tensor.transpose(
                                pt[:, i, :],
                                zrow4[:, rt, cc * 128 : (cc + 1) * 128],
                                identh,
                            )
                        nc.vector.tensor_copy(
                            out=zT[:, ccp * 4 : (ccp + 1) * 4,
                                   rt * 128 : (rt + 1) * 128],
                            in_=pt,
                        )
                    out.append(emit)
            return out

        def emit_v_proj(c):
            zT = zTs[c]
            v_aug = chunkp.tile([128, RT, H, DH + 1], F16, tag="vaug")
            for rt in range(RT):
                for vh in range(2):
                    pv = pv_pool.tile([128, 384], F32, tag="pv")
                    j0 = DM + vh * 384
                    for cc in range(CC):
                        nc.tensor.matmul(
                            pv, zT[:, cc, rt * 128 : (rt + 1) * 128],
                            wkv_sb[:, cc, j0 : j0 + 384],
                            start=(cc == 0), stop=(cc == CC - 1),
                        )
                    nc.vector.tensor_copy(
                        out=v_aug[:, rt, vh * 4 : (vh + 1) * 4, 0:DH],
                        in_=pv.rearrange("p (h d) -> p h d", d=DH),
                    )
            nc.vector.memset(v_aug[:, :, :, DH : DH + 1], 1.0)
            vaugs[c] = v_aug

        def emit_k_head(c, h):
            zT = zTs[c]
            pk = pk_pool.tile([DH, CHUNK], F32, tag="pk")
            for cc in range(CC):
                nc.tensor.matmul(
                    pk, wkv_sb[:, cc, h * DH : (h + 1) * DH], zT[:, cc, :],
                    start=(cc == 0), stop=(cc == CC - 1),
                )
            kT = attp.tile([DH, CHUNK], F16, tag="kT")
            nc.vector.tensor_copy(out=kT, in_=pk)
            return kT

        def emit_sim_pair(c, h, kT, at4, rtp):
            for i in range(2):
                rt = rtp * 2 + i
                ps = ps_pool.tile([128, NQ], F32, tag="ps")
                nc.tensor.matmul(
                    ps, kT[:, rt * 128 : (rt + 1) * 128], qT[:, h, :],
                    start=True, stop=True,
                )
                nc.scalar.activation(
                    out=at4[:, rt, :], in_=ps,
                    func=mybir.ActivationFunctionType.Exp,
                )

        def emit_pv_head(c, h, at4):
            v_aug = vaugs[c]
            pacc = pa_pool.tile([DH + 1, NQ], F32, tag="pa")
            for rt in range(RT):
                nc.tensor.matmul(
                    pacc, v_aug[:, rt, h, :], at4[:, rt, :],
                    start=(rt == 0), stop=(rt == RT - 1),
                )
            nc.vector.tensor_add(out=acc[:, h, :], in0=acc[:, h, :], in1=pacc)

        def emit_finale_head(h):
            # acc[:, h, :] -> per-query normalize -> pooledT[:, h, :]
            for qh in range(2):
                ps = ps_pool.tile([128, NQ], F32, tag="ps")
                nc.tensor.transpose(
                    ps[:, 0 : DH + 1],
                    acc[:, h, qh * 128 : (qh + 1) * 128],
                    identf[0 : DH + 1, 0 : DH + 1],
                )
                rz = work.tile([128, 1], F32, tag="rz")
                nc.vector.reciprocal(out=rz, in_=ps[:, DH : DH + 1])
                pN = work.tile([128, DH], F16, tag="pN")
                nc.vector.tensor_scalar(
                    out=pN, in0=ps[:, 0:DH], scalar1=rz, scalar2=None,
                    op0=mybir.AluOpType.mult,
                )
                pt = pz_pool.tile([128, 4, 128], F16, tag="pz")
                nc.tensor.transpose(pt[0:DH, 0, :], pN, identh)
                nc.vector.tensor_copy(
                    out=pooledT[:, h, qh * 128 : (qh + 1) * 128],
                    in_=pt[0:DH, 0, :],
                )

        for ch in range(N_CHUNKS + 1):
            if ch == 0:
                # fully per-rt pipelined: each row-tile's LN + transposes
                # start the moment its x DMA lands (minimizes startup dead
                # time); rstd via scalar Sqrt per rt
                emit_ln_alloc(0, xts=xts0, fast_rstd=True)
                _, mv_all0, rstd40, _ = ln_state[0]
                slices0 = transpose_slices(0)
                for rt in range(RT):
                    emit_ln_stats(0, rt)
                    xstd1 = work.tile([128, 1], F32, tag="xstd1")
                    nc.scalar.activation(
                        out=xstd1, in_=mv_all0[:, rt, 1:2],
                        func=mybir.ActivationFunctionType.Sqrt, bias=eps_t,
                    )
                    nc.vector.reciprocal(
                        out=rstd40[:, rt : rt + 1], in_=xstd1
                    )
                    emit_ln_zrow(0, rt)
                    slices0[2 * rt]()
                    slices0[2 * rt + 1]()
                continue
            c = ch - 1
            # V projection first so its PSUM drains lead the vector queue
            # (they gate attn@v). The NEXT chunk's LN vector work is dripped
            # into the head loop so kT drains aren't queued behind it.
            emit_v_proj(c)
            if ch < N_CHUNKS:
                emit_ln_alloc(ch, xts=xts1 if ch == 1 else None)
            if ch == 2:
                nc.scalar.dma_start(
                    out=wout_sb, in_=wout.rearrange("(h p) j -> p h j", p=DH)
                )
            filler = transpose_slices(ch) if ch < N_CHUNKS else None
            if filler is None:
                # qh=0 output projection accumulates inline as heads finalize
                po0a = pv_pool.tile([128, 384], F32, tag="pv")
                po0b = pv_pool.tile([128, 384], F32, tag="pv")
                po0 = [po0a, po0b]
            fi = 0
            kTs, at4s = {}, {}
            for h in range(H + 2):
                # K(h) first: gives kT(h-1)'s drain a full PE-burst of slack.
                # sims for h-1 split around pv(h-2) so the psum slots
                # recycle only after the matching Exp has drained them.
                if h < H:
                    kTs[h] = emit_k_head(c, h)
                if 1 <= h <= H:
                    kT_prev = kTs.pop(h - 1)
                    at4 = attp.tile([128, RT, NQ], F16, tag="at")
                    at4s[h - 1] = at4
                    emit_sim_pair(c, h - 1, kT_prev, at4, 0)
                if h >= 2:
                    emit_pv_head(c, h - 2, at4s.pop(h - 2))
                    if filler is None:
                        emit_finale_head(h - 2)
                        for j in range(2):
                            nc.tensor.matmul(
                                po0[j], pooledT[:, h - 2, 0:128],
                                wout_sb[:, h - 2, j * 384 : (j + 1) * 384],
                                start=(h - 2 == 0), stop=(h - 2 == H - 1),
                            )
                if 1 <= h <= H:
                    emit_sim_pair(c, h - 1, kT_prev, at4s[h - 1], 1)
                if filler is not None:
                    # drip next chunk's LN: stats at h<4, rstd at 4, zrow 4-7,
                    # transposes (PE fillers) at 5-8 chasing zrow readiness
                    if h < RT:
                        emit_ln_stats(ch, h)
                    if h == RT:
                        emit_ln_rstd(ch)
                    if RT <= h < 2 * RT:
                        emit_ln_zrow(ch, h - RT)
                    if 5 <= h <= 8:
                        for _ in range(2):
                            if fi < len(filler):
                                filler[fi](); fi += 1
            if filler is not None:
                while fi < len(filler):
                    filler[fi](); fi += 1

        # ---- output projection ----
        ob_bc = None
        if has_out_bias:
            ob = singles.tile([1, DM], F32)
            nc.sync.dma_start(out=ob, in_=out_bias[:, :])
            ob_bc = singles.tile([128, DM], F32)
            nc.gpsimd.partition_broadcast(ob_bc, ob)
        for qh in range(2):
            if qh == 0:
                pj = po0  # accumulated inline during the finale
            else:
                pja = pv_pool.tile([128, 384], F32, tag="pv")
                pjb = pv_pool.tile([128, 384], F32, tag="pv")
                pj = [pja, pjb]
                for h in range(H):
                    for j in range(2):
                        nc.tensor.matmul(
                            pj[j], pooledT[:, h, qh * 128 : (qh + 1) * 128],
                            wout_sb[:, h, j * 384 : (j + 1) * 384],
                            start=(h == 0), stop=(h == H - 1),
                        )
            ot = work.tile([128, DM], F32, tag="ot")
            for j in range(2):
                sl = slice(j * 384, (j + 1) * 384)
                if ob_bc is not None:
                    nc.vector.tensor_add(out=ot[:, sl], in0=pj[j], in1=ob_bc[:, sl])
                else:
                    nc.vector.tensor_copy(out=ot[:, sl], in_=pj[j])
            nc.sync.dma_start(out=out[qh * 128 : (qh + 1) * 128, :], in_=ot)
    nc.compile()
    return nc


_NC_CACHE = {}
_TRACE = False


def kernel(**inputs):
    x = np.asarray(inputs["x"], dtype=np.float32)
    query = np.asarray(inputs["query"], dtype=np.float32)
    ln_k_g = np.asarray(inputs["ln_k_g"], dtype=np.float32)
    ln_k_b = np.asarray(inputs["ln_k_b"], dtype=np.float32)
    ln_q_g = np.asarray(inputs["ln_q_g"], dtype=np.float32)
    ln_q_b = np.asarray(inputs["ln_q_b"], dtype=np.float32)
    W_q = np.asarray(inputs["W_q"], dtype=np.float32)
    W_kv = np.asarray(inputs["W_kv"], dtype=np.float32)
    W_out = np.asarray(inputs["W_out"], dtype=np.float32)

    scale = DH ** -0.5
    wkvg = (ln_k_g[:, None] * W_kv).astype(np.float16)
    wqg = (ln_q_g[:, None] * W_q * scale).astype(np.float16)
    qbias = ((ln_q_b @ W_q) * scale).astype(np.float32)
    kv_bias = ln_k_b @ W_kv
    vb = kv_bias[DM:]
    has_out_bias = bool(np.any(vb != 0.0))
    x16 = x.astype(np.float16)

    key = has_out_bias
    if key not in _NC_CACHE:
        _NC_CACHE[key] = build_nc(has_out_bias)
    nc = _NC_CACHE[key]

    shared = dict(
        query=query.astype(np.float16), wkvg=wkvg, wqg=wqg,
        wout=W_out.astype(np.float16),
        qbias=qbias,
        identh=np.eye(128, dtype=np.float16),
        identf=np.eye(128, dtype=np.float32),
    )
    if has_out_bias:
        shared["out_bias"] = (vb @ W_out).astype(np.float32).reshape(1, DM)
    in_maps = [dict(x=x16[i], **shared) for i in range(N_CORES)]
    res = run_bass_kernel_spmd(
        nc, in_maps, core_ids=list(range(N_CORES)), trace=_TRACE
    )
    kernel.last_result = res
    out = np.stack([np.asarray(res.results[i]["out"]) for i in range(N_CORES)])
    return out.astype(np.float32)


if __name__ == "__main__":
    rng = np.random.default_rng(0)
    ins = {
        "x": rng.standard_normal((B, N, CTX), dtype=np.float32),
        "query": rng.standard_normal((NQ, DM), dtype=np.float32),
        "ln_k_g": np.ones(CTX, np.float32),
        "ln_k_b": np.zeros(CTX, np.float32),
        "ln_q_g": np.ones(DM, np.float32),
        "ln_q_b": np.zeros(DM, np.float32),
        "W_q": rng.standard_normal((DM, DM), dtype=np.float32) * DM ** -0.5,
        "W_kv": rng.standard_normal((CTX, 2 * DM), dtype=np.float32) * CTX ** -0.5,
        "W_out": rng.standard_normal((DM, DM), dtype=np.float32) * DM ** -0.5,
    }
    o = kernel(**ins)
    print("out", o.shape, o.dtype, float(np.abs(o).mean()))
